# revision 8
# baseline (speedup 1.0000x reference)
"""Benes butterfly network (12 layers, N=4096) on 8 Trainium2 NeuronCores.

Self-contained: takes full inputs, shards batch across 8 cores, runs a
Bass/Tile kernel per core, gathers the full output.

Math: reference layer k is a butterfly with span 2^k:
    h[:, j] <- A_k[j] * h[:, j] + B_k[j] * h[:, j ^ 2^k]
(A_k/B_k extracted from the sparse COO (values, idx_in, idx_out)).

Device decomposition per core (batch shard 512, transposed layout
[col-part, batch-free], 32 col-tiles of 128; x pre-transposed + bf16 on
the host; output written as outT [col, batch] bf16 and transposed /
upcast on the host):
  1. phase1: layers 0..8 fused into dense 128x128 bf16 block matrices,
     with layer 9's self-scale A9 and the composed L10*L11 self-scale
     P[t] = A11[t]*A10[t] folded in on the host.  4 matmuls per tile
     into quad PSUM tiles (4 banks), one wide ACT evac per quad.
  2. L9/L10/L11 partner terms via the ratio trick on the P-rescaled
     chain (per tile XOR-partner at distance 4/8/16 tiles):
       C1[v] = E[v]  + r9[v]  * E[v^4]
       C2[u] = C1[u] + r10[u] * C1[u^8]
       out[t] = C2[t] + r11[t] * C2[t^16]
     Most ops are DVE scalar_tensor_tensor; a tunable subset is split
     into [ACT/DVE scaled-copy producer + GpSimd tensor add] to spread
     the elementwise load across three engines.
  3. Tiles are processed class-major (t mod 4) so each class's chain
     and 1MB output DMA complete progressively.
"""
import os
import numpy as np

N = 4096
BATCH = 4096
NLAYERS = 12
NCORES = 8
BSH = BATCH // NCORES      # 512 batch rows per core
T = N // 128               # 32 column tiles

# class-major tile order: all t=c (mod 4) for c in 0..3
TILE_ORDER = [c + 4 * g for c in range(4) for g in range(8)]

# chain-op engine assignment: (level, class, g) -> mode
#   'd'  = DVE scalar_tensor_tensor (1 op)
#   'a+' = ACT scaled-copy producer + GpSimd tensor add
#   'v+' = DVE tensor_scalar producer + GpSimd tensor add
# Later classes stay on the fast DVE path so the end-of-kernel tail is
# short; early classes have slack for the slower GpSimd adds.
def _chain_mode(level, cls, g):
    if level == 9:
        return 'a+' if (cls in (0, 1) and g in (1, 3, 5, 7)) else 'd'
    if level == 10:
        return 'v+' if (cls in (0, 1, 2) and g in (1, 3, 5, 7)) else 'd'
    # level 11
    if cls in (0, 1):
        return 'a+'
    if cls == 2 and g in (1, 3, 5, 7):
        return 'a+'
    return 'd'

_PROGRAM_CACHE = {}
LAST_EXEC_NS = None


def _bf16(a):
    import ml_dtypes
    return np.asarray(a, dtype=ml_dtypes.bfloat16)


def _extract_ab(values, idx_in, idx_out):
    """Per-layer butterfly coefficients A[k], B[k] (float64 [L, N])."""
    v = np.asarray(values, np.float64)
    ii = np.asarray(idx_in, np.int64)
    io = np.asarray(idx_out, np.int64)
    L, nnz = v.shape
    n = nnz // 2
    A = np.zeros((L, n))
    B = np.zeros((L, n))
    for k in range(L):
        s = 1 << k
        self_m = ii[k] == io[k]
        part_m = ii[k] == (io[k] ^ s)
        if not np.all(self_m | part_m):
            raise ValueError(f"layer {k}: unexpected sparse index structure")
        np.add.at(A[k], io[k][self_m], v[k][self_m])
        np.add.at(B[k], io[k][part_m], v[k][part_m])
    return A, B


def _host_precompute(values, idx_in, idx_out):
    A, B = _extract_ab(values, idx_in, idx_out)
    Ab = A.reshape(NLAYERS, T, 128)
    Bb = B.reshape(NLAYERS, T, 128)
    j = np.arange(128)

    # Block-level composition of layers 0..8: S[t] = {src_tile: 128x128}.
    S = [{t: np.eye(128)} for t in range(T)]
    for k in range(7):  # within-block layers
        s = 1 << k
        for t in range(T):
            W = np.zeros((128, 128))
            W[j, j] = Ab[k, t]
            W[j, j ^ s] = Bb[k, t]
            S[t] = {src: W @ M for src, M in S[t].items()}
    for k in (7, 8):   # cross-block layers, tile distance d
        d = 1 << (k - 7)
        newS = []
        for t in range(T):
            out = {}
            for src, M in S[t].items():
                out[src] = Ab[k, t][:, None] * M
            for src, M in S[t ^ d].items():
                out[src] = out.get(src, 0) + Bb[k, t][:, None] * M
            newS.append(out)
        S = newS

    def guard(v):
        return np.where(np.abs(v) < 1e-12, 1e-12, v)

    A9 = guard(Ab[9])
    A10 = guard(Ab[10])
    A11 = guard(Ab[11])
    # P[t] = A11[t]*A10[t]: composed L10/L11 self-scale, folded into mst
    P = A11 * A10                                   # [T, 128]

    # mst packed in class-major tile order, bf16:
    # slice k (tile t=TILE_ORDER[k]) holds 4 transposed stationaries,
    # output columns pre-scaled by A9[t]*P[t].
    mst = np.zeros((128, T * 512), np.float64)
    for k, t in enumerate(TILE_ORDER):
        assert set(S[t].keys()) == {t, t ^ 1, t ^ 2, t ^ 3}
        for ji in range(4):
            M = (P[t] * A9[t])[:, None] * S[t][t ^ ji]
            mst[:, k * 512 + ji * 128: k * 512 + (ji + 1) * 128] = M.T

    # Ratio scalars for the P-rescaled chain:
    xt = lambda v, d: v[np.arange(T) ^ d]           # tile-XOR view
    r9 = P * (Bb[9] / xt(A9, 4)) / xt(P, 4)
    r10 = P * (Bb[10] / A10) / xt(P, 8)
    r11 = P * (Bb[11] * xt(A10, 16) / (A11 * A10)) / xt(P, 16)

    # consts [128, 96] fp32: cols 0-31 r9, 32-63 r10, 64-95 r11
    consts = np.zeros((128, 96), np.float32)
    consts[:, 0:32] = r9.T
    consts[:, 32:64] = r10.T
    consts[:, 64:96] = r11.T
    return _bf16(mst), consts


def _build_program():
    import concourse.bass as bass
    import concourse.mybir as mybir
    import concourse.tile as tile
    from concourse import bacc

    f32 = mybir.dt.float32
    bf16 = mybir.dt.bfloat16
    mult = mybir.AluOpType.mult
    add = mybir.AluOpType.add

    nc = bacc.Bacc("TRN2", target_bir_lowering=False, debug=False)
    # x pre-transposed + bf16 on host: [N, BSH] (column-major over batch)
    xT_ap = nc.dram_tensor("xT", [N, BSH], bf16, kind="ExternalInput").ap()
    mst_ap = nc.dram_tensor("mst", [128, T * 512], bf16, kind="ExternalInput").ap()
    cst_ap = nc.dram_tensor("consts", [128, 96], f32, kind="ExternalInput").ap()
    out_ap = nc.dram_tensor("outT", [N, BSH], bf16, kind="ExternalOutput").ap()

    with tile.TileContext(nc) as tc:
        with (
            tc.tile_pool(name="const", bufs=1) as constp,
            tc.tile_pool(name="h0", bufs=8) as h0p,
            tc.tile_pool(name="mstp", bufs=4) as mstp,
            tc.tile_pool(name="E", bufs=4) as Ep,
            tc.tile_pool(name="C1", bufs=16) as C1p,
            tc.tile_pool(name="C2", bufs=16) as C2p,
            tc.tile_pool(name="scr", bufs=10) as scrp,
            tc.tile_pool(name="ost", bufs=4) as ostp,
            tc.tile_pool(name="ps", bufs=2, space="PSUM") as psp,
        ):
            cst = constp.tile([128, 96], f32)
            nc.scalar.dma_start(cst[:], cst_ap[:])
            r_sc = {9: cst[:, 0:32], 10: cst[:, 32:64], 11: cst[:, 64:96]}

            # mst in 8 chunks of 4 class-major slices (512KB each)
            msts = {}
            for mb in range(8):
                mt = mstp.tile([128, 2048], bf16, tag="mst", name=f"mst_{mb}")
                nc.scalar.dma_start(mt[:], mst_ap[:, mb * 2048:(mb + 1) * 2048])
                msts[mb] = mt

            # H0 tiles via 512KB 3D-strided DMAs, resident for all passes:
            # H0cat[kb][p, lt*512+b] = xT[(4*kb+lt)*128 + p, b]
            H0cat = {}
            for kb in range(8):
                h0c = h0p.tile([128, 2048], bf16, tag="h0", name=f"h0c_{kb}")
                src = xT_ap[kb * 512:(kb + 1) * 512, :].rearrange(
                    "(lt p) b -> p lt b", lt=4, p=128
                )
                nc.sync.dma_start(h0c[:].rearrange("p (lt b) -> p lt b", lt=4), src)
                H0cat[kb] = h0c

            def chain_op(level, tt, out_slice, partner, self_, scal):
                """out = self_ + scal * partner, engine per _chain_mode."""
                mode = _chain_mode(level, tt % 4, tt // 4)
                if mode == 'd':
                    nc.vector.scalar_tensor_tensor(
                        out_slice, partner, scal, self_, op0=mult, op1=add,
                    )
                else:
                    s = scrp.tile([128, 512], bf16, tag="scr")
                    if mode == 'a+':
                        nc.scalar.mul(s[:], partner, scal)
                    else:
                        nc.vector.tensor_scalar_mul(s[:], partner, scal)
                    nc.gpsimd.tensor_add(out_slice, s[:], self_)

            E, C1, C2 = {}, {}, {}
            for k4 in range(8):          # quad index over TILE_ORDER
                quad = TILE_ORDER[k4 * 4:(k4 + 1) * 4]
                p1q = psp.tile([128, 2048], f32, name=f"p1q_{k4}", tag="ps")
                for kk, t in enumerate(quad):
                    qt, lt = t // 4, t % 4
                    mchunk = msts[(k4 * 4 + kk) // 4]
                    moff = kk * 512
                    for ji in range(4):
                        nc.tensor.matmul(
                            p1q[:, kk * 512:(kk + 1) * 512],
                            mchunk[:, moff + ji * 128: moff + (ji + 1) * 128],
                            H0cat[qt][:, (lt ^ ji) * 512:((lt ^ ji) + 1) * 512],
                            start=(ji == 0), stop=(ji == 3),
                        )
                # one wide ACT evac per quad (P folded into mst on host)
                eq = Ep.tile([128, 2048], bf16, tag="E", name=f"Eq_{k4}")
                nc.scalar.copy(eq[:], p1q[:])
                for kk, t in enumerate(quad):
                    E[t] = eq[:, kk * 512:(kk + 1) * 512]

                cls, g = quad[-1] % 4, quad[-1] // 4
                if g == 7:
                    # class complete: run its whole L9/L10/L11 chain
                    tls = [cls + 4 * gg for gg in range(8)]
                    for v in tls:
                        C1[v] = C1p.tile([128, 512], bf16, tag="C1", name=f"C1_{v}")
                        chain_op(9, v, C1[v][:], E[v ^ 4], E[v],
                                 r_sc[9][:, v:v + 1])
                    for u in tls:
                        C2[u] = C2p.tile([128, 512], bf16, tag="C2", name=f"C2_{u}")
                        chain_op(10, u, C2[u][:], C1[u ^ 8][:], C1[u][:],
                                 r_sc[10][:, u:u + 1])
                    oc = ostp.tile([128, 4096], bf16, tag="ost", name=f"ost_{cls}")
                    for gg, tt in enumerate(tls):
                        chain_op(11, tt, oc[:, gg * 512:(gg + 1) * 512],
                                 C2[tt ^ 16][:], C2[tt][:],
                                 r_sc[11][:, tt:tt + 1])
                    # outT rows (cls + 4g)*128 + p  <- oc[:, g*512 + b]
                    dst = out_ap.rearrange(
                        "(g c p) b -> p c g b", g=8, c=4, p=128
                    )[:, cls, :, :]
                    nc.sync.dma_start(dst, oc[:].rearrange("p (g b) -> p g b", g=8))

    nc.compile()
    return nc


def kernel(x, values, idx_in, idx_out):
    global LAST_EXEC_NS
    from concourse.bass_utils import run_bass_kernel_spmd

    x = np.asarray(x, np.float32)
    assert x.shape == (BATCH, N), x.shape
    mst, consts = _host_precompute(values, idx_in, idx_out)
    xT = _bf16(np.ascontiguousarray(x.T))

    if "prog" not in _PROGRAM_CACHE:
        _PROGRAM_CACHE["prog"] = _build_program()
    nc = _PROGRAM_CACHE["prog"]

    in_maps = [
        {
            "xT": np.ascontiguousarray(xT[:, i * BSH:(i + 1) * BSH]),
            "mst": mst,
            "consts": consts,
        }
        for i in range(NCORES)
    ]
    res = run_bass_kernel_spmd(nc, in_maps, core_ids=list(range(NCORES)))
    if os.environ.get("BENES_TRACE"):
        tres = run_bass_kernel_spmd(
            nc, in_maps, core_ids=list(range(NCORES)), trace=True
        )
        LAST_EXEC_NS = tres.exec_time_ns
        _PROGRAM_CACHE["profile_json"] = tres.profile_json
    out = np.empty((BATCH, N), np.float32)
    for i in range(NCORES):
        out[i * BSH:(i + 1) * BSH] = np.asarray(
            res.results[i]["outT"], np.float32
        ).T
    return out


# revision 9
# speedup vs baseline: 1.0742x; 1.0742x over previous
"""Benes butterfly network (12 layers, N=4096) on 8 Trainium2 NeuronCores.

Self-contained: takes full inputs, shards batch across 8 cores, runs a
Bass/Tile kernel per core, gathers the full output.

Math: reference layer k is a butterfly with span 2^k:
    h[:, j] <- A_k[j] * h[:, j] + B_k[j] * h[:, j ^ 2^k]
(A_k/B_k extracted from the sparse COO (values, idx_in, idx_out)).

Device decomposition per core (batch shard 512, transposed layout
[col-part, batch-free], 32 col-tiles of 128; x pre-transposed + bf16 on
the host; output written as outT [col, batch] bf16 and transposed /
upcast on the host):
  1. phase1: layers 0..8 fused into dense 128x128 bf16 block matrices,
     with layer 9's self-scale A9 and the composed L10*L11 self-scale
     P[t] = A11[t]*A10[t] folded in on the host.  4 matmuls per tile
     into quad PSUM tiles (4 banks, 4 class-major tiles each).
  2. One wide ACT evac per quad -> E, then layer 9's partner term as a
     5th accumulating matmul per tile: psum[t] += diag(r9[t]) @ E[t^4]
     (t^4 lives in the same quad).  Second wide evac -> C1.
  3. L10/L11 partner terms as DVE scalar_tensor_tensor on the
     P-rescaled ratio chain:
       C2[u]  = C1[u] + r10[u] * C1[u^8]      (u^8 in the same quad)
       out[t] = C2[t] + r11[t] * C2[t^16]     (t^16 in the other quad)
  4. Tiles are processed class-major (t mod 4) so each class's chain
     and 1MB output DMA complete progressively.
"""
import os
import numpy as np

N = 4096
BATCH = 4096
NLAYERS = 12
NCORES = 8
BSH = BATCH // NCORES      # 512 batch rows per core
T = N // 128               # 32 column tiles

# class-major tile order: all t=c (mod 4) for c in 0..3
TILE_ORDER = [c + 4 * g for c in range(4) for g in range(8)]

_PROGRAM_CACHE = {}
LAST_EXEC_NS = None


def _bf16(a):
    import ml_dtypes
    return np.asarray(a, dtype=ml_dtypes.bfloat16)


def _extract_ab(values, idx_in, idx_out):
    """Per-layer butterfly coefficients A[k], B[k] (float64 [L, N])."""
    v = np.asarray(values, np.float64)
    ii = np.asarray(idx_in, np.int64)
    io = np.asarray(idx_out, np.int64)
    L, nnz = v.shape
    n = nnz // 2
    A = np.zeros((L, n))
    B = np.zeros((L, n))
    for k in range(L):
        s = 1 << k
        self_m = ii[k] == io[k]
        part_m = ii[k] == (io[k] ^ s)
        if not np.all(self_m | part_m):
            raise ValueError(f"layer {k}: unexpected sparse index structure")
        np.add.at(A[k], io[k][self_m], v[k][self_m])
        np.add.at(B[k], io[k][part_m], v[k][part_m])
    return A, B


def _host_precompute(values, idx_in, idx_out):
    A, B = _extract_ab(values, idx_in, idx_out)
    Ab = A.reshape(NLAYERS, T, 128)
    Bb = B.reshape(NLAYERS, T, 128)
    j = np.arange(128)

    # Block-level composition of layers 0..8: S[t] = {src_tile: 128x128}.
    S = [{t: np.eye(128)} for t in range(T)]
    for k in range(7):  # within-block layers
        s = 1 << k
        for t in range(T):
            W = np.zeros((128, 128))
            W[j, j] = Ab[k, t]
            W[j, j ^ s] = Bb[k, t]
            S[t] = {src: W @ M for src, M in S[t].items()}
    for k in (7, 8):   # cross-block layers, tile distance d
        d = 1 << (k - 7)
        newS = []
        for t in range(T):
            out = {}
            for src, M in S[t].items():
                out[src] = Ab[k, t][:, None] * M
            for src, M in S[t ^ d].items():
                out[src] = out.get(src, 0) + Bb[k, t][:, None] * M
            newS.append(out)
        S = newS

    def guard(v):
        return np.where(np.abs(v) < 1e-12, 1e-12, v)

    A9 = guard(Ab[9])
    A10 = guard(Ab[10])
    A11 = guard(Ab[11])
    # P[t] = A11[t]*A10[t]: composed L10/L11 self-scale, folded into mst
    P = A11 * A10                                   # [T, 128]

    # mst packed in class-major tile order, bf16:
    # slice k (tile t=TILE_ORDER[k]) holds 4 transposed stationaries,
    # output columns pre-scaled by A9[t]*P[t].
    mst = np.zeros((128, T * 512), np.float64)
    for k, t in enumerate(TILE_ORDER):
        assert set(S[t].keys()) == {t, t ^ 1, t ^ 2, t ^ 3}
        for ji in range(4):
            M = (P[t] * A9[t])[:, None] * S[t][t ^ ji]
            mst[:, k * 512 + ji * 128: k * 512 + (ji + 1) * 128] = M.T

    # Ratio scalars for the P-rescaled chain:
    xt = lambda v, d: v[np.arange(T) ^ d]           # tile-XOR view
    r9 = P * (Bb[9] / xt(A9, 4)) / xt(P, 4)
    r10 = P * (Bb[10] / A10) / xt(P, 8)
    r11 = P * (Bb[11] * xt(A10, 16) / (A11 * A10)) / xt(P, 16)

    # diag(r9[t]) stationaries, class-major packing [128, 32*128] bf16
    rdiag = np.zeros((128, T * 128), np.float64)
    for k, t in enumerate(TILE_ORDER):
        rdiag[j, k * 128 + j] = r9[t]

    # consts [128, 64] fp32: cols 0-31 r10, 32-63 r11
    consts = np.zeros((128, 64), np.float32)
    consts[:, 0:32] = r10.T
    consts[:, 32:64] = r11.T
    return _bf16(mst), _bf16(rdiag), consts


def _build_program():
    import concourse.bass as bass
    import concourse.mybir as mybir
    import concourse.tile as tile
    from concourse import bacc

    f32 = mybir.dt.float32
    bf16 = mybir.dt.bfloat16
    mult = mybir.AluOpType.mult
    add = mybir.AluOpType.add

    nc = bacc.Bacc("TRN2", target_bir_lowering=False, debug=False)
    # x pre-transposed + bf16 on host: [N, BSH] (column-major over batch)
    xT_ap = nc.dram_tensor("xT", [N, BSH], bf16, kind="ExternalInput").ap()
    mst_ap = nc.dram_tensor("mst", [128, T * 512], bf16, kind="ExternalInput").ap()
    rd_ap = nc.dram_tensor("rdiag", [128, T * 128], bf16, kind="ExternalInput").ap()
    cst_ap = nc.dram_tensor("consts", [128, 64], f32, kind="ExternalInput").ap()
    out_ap = nc.dram_tensor("outT", [N, BSH], bf16, kind="ExternalOutput").ap()

    with tile.TileContext(nc) as tc:
        with (
            tc.tile_pool(name="const", bufs=1) as constp,
            tc.tile_pool(name="h0", bufs=8) as h0p,
            tc.tile_pool(name="mstp", bufs=4) as mstp,
            tc.tile_pool(name="rdp", bufs=2) as rdp,
            tc.tile_pool(name="E", bufs=3) as Ep,
            tc.tile_pool(name="C1", bufs=3) as C1p,
            tc.tile_pool(name="C2", bufs=3) as C2p,
            tc.tile_pool(name="ost", bufs=3) as ostp,
            tc.tile_pool(name="ps", bufs=2, space="PSUM") as psp,
        ):
            cst = constp.tile([128, 64], f32)
            nc.scalar.dma_start(cst[:], cst_ap[:])
            r10_sc = cst[:, 0:32]
            r11_sc = cst[:, 32:64]

            # mst in 8 chunks of 4 class-major slices (512KB each)
            msts = {}
            for mb in range(8):
                mt = mstp.tile([128, 2048], bf16, tag="mst", name=f"mst_{mb}")
                nc.scalar.dma_start(mt[:], mst_ap[:, mb * 2048:(mb + 1) * 2048])
                msts[mb] = mt
            # diag(r9) stationaries in 2 chunks (512KB each)
            rds = {}
            for rb in range(2):
                rt = rdp.tile([128, 2048], bf16, tag="rd", name=f"rd_{rb}")
                nc.scalar.dma_start(rt[:], rd_ap[:, rb * 2048:(rb + 1) * 2048])
                rds[rb] = rt

            # H0 tiles via 512KB 3D-strided DMAs, resident for all passes:
            # H0cat[kb][p, lt*512+b] = xT[(4*kb+lt)*128 + p, b]
            H0cat = {}
            for kb in range(8):
                h0c = h0p.tile([128, 2048], bf16, tag="h0", name=f"h0c_{kb}")
                src = xT_ap[kb * 512:(kb + 1) * 512, :].rearrange(
                    "(lt p) b -> p lt b", lt=4, p=128
                )
                nc.sync.dma_start(h0c[:].rearrange("p (lt b) -> p lt b", lt=4), src)
                H0cat[kb] = h0c

            E, C1, C2 = {}, {}, {}
            C2q = {}
            for k4 in range(8):          # quad index over TILE_ORDER
                quad = TILE_ORDER[k4 * 4:(k4 + 1) * 4]
                p1q = psp.tile([128, 2048], f32, name=f"p1q_{k4}", tag="ps")
                for kk, t in enumerate(quad):
                    qt, lt = t // 4, t % 4
                    mchunk = msts[(k4 * 4 + kk) // 4]
                    moff = kk * 512
                    for ji in range(4):
                        nc.tensor.matmul(
                            p1q[:, kk * 512:(kk + 1) * 512],
                            mchunk[:, moff + ji * 128: moff + (ji + 1) * 128],
                            H0cat[qt][:, (lt ^ ji) * 512:((lt ^ ji) + 1) * 512],
                            start=(ji == 0), stop=False,
                        )
                # wide ACT evac of the phase-1 results (pre-L9 state)
                eq = Ep.tile([128, 2048], bf16, tag="E", name=f"Eq_{k4}")
                nc.scalar.copy(eq[:], p1q[:])
                for kk, t in enumerate(quad):
                    E[t] = eq[:, kk * 512:(kk + 1) * 512]
                # L9 partner: psum[t] += diag(r9[t]) @ E[t^4], same quad
                # (in-quad position of t^4 is kk^1)
                for kk, t in enumerate(quad):
                    k = k4 * 4 + kk
                    nc.tensor.matmul(
                        p1q[:, kk * 512:(kk + 1) * 512],
                        rds[k // 16][:, (k % 16) * 128:(k % 16 + 1) * 128],
                        E[t ^ 4],
                        start=False, stop=True,
                    )
                # second wide evac -> C1 (post-L9 state)
                cq = C1p.tile([128, 2048], bf16, tag="C1", name=f"C1q_{k4}")
                nc.scalar.copy(cq[:], p1q[:])
                for kk, t in enumerate(quad):
                    C1[t] = cq[:, kk * 512:(kk + 1) * 512]

                # L10 within the quad: u^8 is in-quad position kk^2
                c2 = C2p.tile([128, 2048], bf16, tag="C2", name=f"C2q_{k4}")
                C2q[k4] = c2
                for kk, u in enumerate(quad):
                    C2[u] = c2[:, kk * 512:(kk + 1) * 512]
                    nc.vector.scalar_tensor_tensor(
                        C2[u], C1[u ^ 8], r10_sc[:, u:u + 1], C1[u],
                        op0=mult, op1=add,
                    )

                cls, g = quad[-1] % 4, quad[-1] // 4
                if g == 7:
                    # both quads of this class done: L11 + output
                    tls = [cls + 4 * gg for gg in range(8)]
                    oc = ostp.tile([128, 4096], bf16, tag="ost", name=f"ost_{cls}")
                    for gg, tt in enumerate(tls):
                        nc.vector.scalar_tensor_tensor(
                            oc[:, gg * 512:(gg + 1) * 512],
                            C2[tt ^ 16], r11_sc[:, tt:tt + 1], C2[tt],
                            op0=mult, op1=add,
                        )
                    # outT rows (cls + 4g)*128 + p  <- oc[:, g*512 + b]
                    dst = out_ap.rearrange(
                        "(g c p) b -> p c g b", g=8, c=4, p=128
                    )[:, cls, :, :]
                    nc.sync.dma_start(dst, oc[:].rearrange("p (g b) -> p g b", g=8))

    nc.compile()
    return nc


def kernel(x, values, idx_in, idx_out):
    global LAST_EXEC_NS
    from concourse.bass_utils import run_bass_kernel_spmd

    x = np.asarray(x, np.float32)
    assert x.shape == (BATCH, N), x.shape
    mst, rdiag, consts = _host_precompute(values, idx_in, idx_out)
    xT = _bf16(np.ascontiguousarray(x.T))

    if "prog" not in _PROGRAM_CACHE:
        _PROGRAM_CACHE["prog"] = _build_program()
    nc = _PROGRAM_CACHE["prog"]

    in_maps = [
        {
            "xT": np.ascontiguousarray(xT[:, i * BSH:(i + 1) * BSH]),
            "mst": mst,
            "rdiag": rdiag,
            "consts": consts,
        }
        for i in range(NCORES)
    ]
    res = run_bass_kernel_spmd(nc, in_maps, core_ids=list(range(NCORES)))
    if os.environ.get("BENES_TRACE"):
        tres = run_bass_kernel_spmd(
            nc, in_maps, core_ids=list(range(NCORES)), trace=True
        )
        LAST_EXEC_NS = tres.exec_time_ns
        _PROGRAM_CACHE["profile_json"] = tres.profile_json
    out = np.empty((BATCH, N), np.float32)
    for i in range(NCORES):
        out[i * BSH:(i + 1) * BSH] = np.asarray(
            res.results[i]["outT"], np.float32
        ).T
    return out


# revision 11
# speedup vs baseline: 1.1330x; 1.0547x over previous
"""Benes butterfly network (12 layers, N=4096) on 8 Trainium2 NeuronCores.

Self-contained: takes full inputs, shards batch across 8 cores, runs a
Bass/Tile kernel per core, gathers the full output.

Math: reference layer k is a butterfly with span 2^k:
    h[:, j] <- A_k[j] * h[:, j] + B_k[j] * h[:, j ^ 2^k]
(A_k/B_k extracted from the sparse COO (values, idx_in, idx_out)).

Device decomposition per core (batch shard 512, transposed layout
[col-part, batch-free], 32 col-tiles of 128; x pre-transposed + bf16 on
the host; output written as outT [col, batch] bf16 and transposed /
upcast on the host):
  1. phase1: layers 0..8 fused into dense 128x128 bf16 block matrices,
     with layer 9's self-scale A9 and the composed L10*L11 self-scale
     P[t] = A11[t]*A10[t] folded in on the host.  4 matmuls per tile
     into quad PSUM tiles (4 banks, 4 class-major tiles each).
  2. One wide ACT evac per quad -> E, then layer 9's partner term as a
     5th accumulating matmul per tile: psum[t] += diag(r9[t]) @ E[t^4]
     (t^4 lives in the same quad).  Second wide evac -> C1.
  3. L10/L11 partner terms as DVE scalar_tensor_tensor on the
     P-rescaled ratio chain:
       C2[u]  = C1[u] + r10[u] * C1[u^8]      (u^8 in the same quad)
       out[t] = C2[t] + r11[t] * C2[t^16]     (t^16 in the other quad)
  4. Tiles are processed class-major (t mod 4) so each class's chain
     and 1MB output DMA complete progressively.
"""
import os
import numpy as np

N = 4096
BATCH = 4096
NLAYERS = 12
NCORES = 8
BSH = BATCH // NCORES      # 512 batch rows per core
T = N // 128               # 32 column tiles

# class-major tile order: all t=c (mod 4) for c in 0..3
TILE_ORDER = [c + 4 * g for c in range(4) for g in range(8)]

_PROGRAM_CACHE = {}
LAST_EXEC_NS = None


def _bf16(a):
    import ml_dtypes
    return np.asarray(a, dtype=ml_dtypes.bfloat16)


def _extract_ab(values, idx_in, idx_out):
    """Per-layer butterfly coefficients A[k], B[k] (float64 [L, N])."""
    v = np.asarray(values, np.float64)
    ii = np.asarray(idx_in, np.int64)
    io = np.asarray(idx_out, np.int64)
    L, nnz = v.shape
    n = nnz // 2
    A = np.zeros((L, n))
    B = np.zeros((L, n))
    for k in range(L):
        s = 1 << k
        self_m = ii[k] == io[k]
        part_m = ii[k] == (io[k] ^ s)
        if not np.all(self_m | part_m):
            raise ValueError(f"layer {k}: unexpected sparse index structure")
        np.add.at(A[k], io[k][self_m], v[k][self_m])
        np.add.at(B[k], io[k][part_m], v[k][part_m])
    return A, B


def _host_precompute(values, idx_in, idx_out):
    A, B = _extract_ab(values, idx_in, idx_out)
    Ab = A.reshape(NLAYERS, T, 128)
    Bb = B.reshape(NLAYERS, T, 128)
    j = np.arange(128)

    # Block-level composition of layers 0..8: S[t] = {src_tile: 128x128}.
    S = [{t: np.eye(128)} for t in range(T)]
    for k in range(7):  # within-block layers
        s = 1 << k
        for t in range(T):
            W = np.zeros((128, 128))
            W[j, j] = Ab[k, t]
            W[j, j ^ s] = Bb[k, t]
            S[t] = {src: W @ M for src, M in S[t].items()}
    for k in (7, 8):   # cross-block layers, tile distance d
        d = 1 << (k - 7)
        newS = []
        for t in range(T):
            out = {}
            for src, M in S[t].items():
                out[src] = Ab[k, t][:, None] * M
            for src, M in S[t ^ d].items():
                out[src] = out.get(src, 0) + Bb[k, t][:, None] * M
            newS.append(out)
        S = newS

    def guard(v):
        return np.where(np.abs(v) < 1e-12, 1e-12, v)

    A9 = guard(Ab[9])
    A10 = guard(Ab[10])
    A11 = guard(Ab[11])
    # P[t] = A11[t]*A10[t]: composed L10/L11 self-scale, folded into mst
    P = A11 * A10                                   # [T, 128]

    # mst packed in class-major tile order, bf16:
    # slice k (tile t=TILE_ORDER[k]) holds 4 transposed stationaries,
    # output columns pre-scaled by A9[t]*P[t].
    mst = np.zeros((128, T * 512), np.float64)
    for k, t in enumerate(TILE_ORDER):
        assert set(S[t].keys()) == {t, t ^ 1, t ^ 2, t ^ 3}
        for ji in range(4):
            M = (P[t] * A9[t])[:, None] * S[t][t ^ ji]
            mst[:, k * 512 + ji * 128: k * 512 + (ji + 1) * 128] = M.T

    # Ratio scalars for the P-rescaled chain:
    xt = lambda v, d: v[np.arange(T) ^ d]           # tile-XOR view
    r9 = P * (Bb[9] / xt(A9, 4)) / xt(P, 4)
    r10 = P * (Bb[10] / A10) / xt(P, 8)
    r11 = P * (Bb[11] * xt(A10, 16) / (A11 * A10)) / xt(P, 16)

    # diag(r9[t]) stationaries, class-major packing [128, 32*128] bf16
    rdiag = np.zeros((128, T * 128), np.float64)
    for k, t in enumerate(TILE_ORDER):
        rdiag[j, k * 128 + j] = r9[t]

    # consts [128, 64] fp32: cols 0-31 r10, 32-63 r11
    consts = np.zeros((128, 64), np.float32)
    consts[:, 0:32] = r10.T
    consts[:, 32:64] = r11.T
    return _bf16(mst), _bf16(rdiag), consts


def _build_program():
    import concourse.bass as bass
    import concourse.mybir as mybir
    import concourse.tile as tile
    from concourse import bacc

    f32 = mybir.dt.float32
    bf16 = mybir.dt.bfloat16
    mult = mybir.AluOpType.mult
    add = mybir.AluOpType.add

    nc = bacc.Bacc("TRN2", target_bir_lowering=False, debug=False)
    # x pre-transposed + bf16 on host: [N, BSH] (column-major over batch)
    xT_ap = nc.dram_tensor("xT", [N, BSH], bf16, kind="ExternalInput").ap()
    mst_ap = nc.dram_tensor("mst", [128, T * 512], bf16, kind="ExternalInput").ap()
    rd_ap = nc.dram_tensor("rdiag", [128, T * 128], bf16, kind="ExternalInput").ap()
    cst_ap = nc.dram_tensor("consts", [128, 64], f32, kind="ExternalInput").ap()
    out_ap = nc.dram_tensor("outT", [N, BSH], bf16, kind="ExternalOutput").ap()

    with tile.TileContext(nc) as tc:
        with (
            tc.tile_pool(name="const", bufs=1) as constp,
            tc.tile_pool(name="h0", bufs=8) as h0p,
            tc.tile_pool(name="mstp", bufs=4) as mstp,
            tc.tile_pool(name="rdp", bufs=2) as rdp,
            tc.tile_pool(name="E", bufs=3) as Ep,
            tc.tile_pool(name="C1", bufs=3) as C1p,
            tc.tile_pool(name="C2", bufs=3) as C2p,
            tc.tile_pool(name="ost", bufs=3) as ostp,
            tc.tile_pool(name="ps", bufs=2, space="PSUM") as psp,
        ):
            cst = constp.tile([128, 64], f32)
            nc.scalar.dma_start(cst[:], cst_ap[:])
            r10_sc = cst[:, 0:32]
            r11_sc = cst[:, 32:64]

            # mst in 8 chunks of 4 class-major slices (512KB each)
            msts = {}
            for mb in range(8):
                mt = mstp.tile([128, 2048], bf16, tag="mst", name=f"mst_{mb}")
                nc.scalar.dma_start(mt[:], mst_ap[:, mb * 2048:(mb + 1) * 2048])
                msts[mb] = mt
            # diag(r9) stationaries in 2 chunks (512KB each)
            rds = {}
            for rb in range(2):
                rt = rdp.tile([128, 2048], bf16, tag="rd", name=f"rd_{rb}")
                nc.scalar.dma_start(rt[:], rd_ap[:, rb * 2048:(rb + 1) * 2048])
                rds[rb] = rt

            # H0 tiles via 512KB 3D-strided DMAs, resident for all passes:
            # H0cat[kb][p, lt*512+b] = xT[(4*kb+lt)*128 + p, b]
            H0cat = {}
            for kb in range(8):
                h0c = h0p.tile([128, 2048], bf16, tag="h0", name=f"h0c_{kb}")
                src = xT_ap[kb * 512:(kb + 1) * 512, :].rearrange(
                    "(lt p) b -> p lt b", lt=4, p=128
                )
                nc.sync.dma_start(h0c[:].rearrange("p (lt b) -> p lt b", lt=4), src)
                H0cat[kb] = h0c

            E, C1, C2 = {}, {}, {}
            C2q = {}
            for k4 in range(8):          # quad index over TILE_ORDER
                quad = TILE_ORDER[k4 * 4:(k4 + 1) * 4]
                p1q = psp.tile([128, 2048], f32, name=f"p1q_{k4}", tag="ps")
                for kk, t in enumerate(quad):
                    qt, lt = t // 4, t % 4
                    mchunk = msts[(k4 * 4 + kk) // 4]
                    moff = kk * 512
                    for ji in range(4):
                        nc.tensor.matmul(
                            p1q[:, kk * 512:(kk + 1) * 512],
                            mchunk[:, moff + ji * 128: moff + (ji + 1) * 128],
                            H0cat[qt][:, (lt ^ ji) * 512:((lt ^ ji) + 1) * 512],
                            start=(ji == 0), stop=False,
                        )
                # wide ACT evac of the phase-1 results (pre-L9 state)
                eq = Ep.tile([128, 2048], bf16, tag="E", name=f"Eq_{k4}")
                nc.scalar.copy(eq[:], p1q[:])
                for kk, t in enumerate(quad):
                    E[t] = eq[:, kk * 512:(kk + 1) * 512]
                # L9 partner: psum[t] += diag(r9[t]) @ E[t^4], same quad
                # (in-quad position of t^4 is kk^1)
                for kk, t in enumerate(quad):
                    k = k4 * 4 + kk
                    nc.tensor.matmul(
                        p1q[:, kk * 512:(kk + 1) * 512],
                        rds[k // 16][:, (k % 16) * 128:(k % 16 + 1) * 128],
                        E[t ^ 4],
                        start=False, stop=True,
                    )
                # L10 with one PSUM operand per op (stt can read only one
                # non-scalar input from PSUM): evac the quad's upper half
                # (in-quad slices 2,3 = the L10 partners of slices 0,1) to
                # SBUF, then pair each psum slice with an SBUF slice.
                c1h = C1p.tile([128, 1024], bf16, tag="C1", name=f"C1h_{k4}")
                nc.scalar.copy(c1h[:], p1q[:, 1024:2048])
                c2 = C2p.tile([128, 2048], bf16, tag="C2", name=f"C2q_{k4}")
                C2q[k4] = c2
                for kk, u in enumerate(quad):
                    C2[u] = c2[:, kk * 512:(kk + 1) * 512]
                    pp = kk ^ 2  # in-quad position of the L10 partner u^8
                    if kk < 2:
                        part = c1h[:, (pp - 2) * 512:(pp - 1) * 512]
                        self_ = p1q[:, kk * 512:(kk + 1) * 512]
                    else:
                        part = p1q[:, pp * 512:(pp + 1) * 512]
                        self_ = c1h[:, (kk - 2) * 512:(kk - 1) * 512]
                    nc.vector.scalar_tensor_tensor(
                        C2[u], part, r10_sc[:, u:u + 1], self_,
                        op0=mult, op1=add,
                    )

                cls, g = quad[-1] % 4, quad[-1] // 4
                if g == 7:
                    # both quads of this class done: L11 + output
                    tls = [cls + 4 * gg for gg in range(8)]
                    oc = ostp.tile([128, 4096], bf16, tag="ost", name=f"ost_{cls}")
                    for gg, tt in enumerate(tls):
                        nc.vector.scalar_tensor_tensor(
                            oc[:, gg * 512:(gg + 1) * 512],
                            C2[tt ^ 16], r11_sc[:, tt:tt + 1], C2[tt],
                            op0=mult, op1=add,
                        )
                    # outT rows (cls + 4g)*128 + p  <- oc[:, g*512 + b]
                    dst = out_ap.rearrange(
                        "(g c p) b -> p c g b", g=8, c=4, p=128
                    )[:, cls, :, :]
                    nc.sync.dma_start(dst, oc[:].rearrange("p (g b) -> p g b", g=8))

    nc.compile()
    return nc


def kernel(x, values, idx_in, idx_out):
    global LAST_EXEC_NS
    from concourse.bass_utils import run_bass_kernel_spmd

    x = np.asarray(x, np.float32)
    assert x.shape == (BATCH, N), x.shape
    mst, rdiag, consts = _host_precompute(values, idx_in, idx_out)
    xT = _bf16(np.ascontiguousarray(x.T))

    if "prog" not in _PROGRAM_CACHE:
        _PROGRAM_CACHE["prog"] = _build_program()
    nc = _PROGRAM_CACHE["prog"]

    in_maps = [
        {
            "xT": np.ascontiguousarray(xT[:, i * BSH:(i + 1) * BSH]),
            "mst": mst,
            "rdiag": rdiag,
            "consts": consts,
        }
        for i in range(NCORES)
    ]
    res = run_bass_kernel_spmd(nc, in_maps, core_ids=list(range(NCORES)))
    if os.environ.get("BENES_TRACE"):
        tres = run_bass_kernel_spmd(
            nc, in_maps, core_ids=list(range(NCORES)), trace=True
        )
        LAST_EXEC_NS = tres.exec_time_ns
        _PROGRAM_CACHE["profile_json"] = tres.profile_json
    out = np.empty((BATCH, N), np.float32)
    for i in range(NCORES):
        out[i * BSH:(i + 1) * BSH] = np.asarray(
            res.results[i]["outT"], np.float32
        ).T
    return out


# revision 14
# speedup vs baseline: 1.1637x; 1.0271x over previous
"""Benes butterfly network (12 layers, N=4096) on 8 Trainium2 NeuronCores.

Self-contained: takes full inputs, shards batch across 8 cores, runs a
Bass/Tile kernel per core, gathers the full output.

Math: reference layer k is a butterfly with span 2^k:
    h[:, j] <- A_k[j] * h[:, j] + B_k[j] * h[:, j ^ 2^k]
(A_k/B_k extracted from the sparse COO (values, idx_in, idx_out)).

Device decomposition per core (batch shard 512, transposed layout
[col-part, batch-free], 32 col-tiles of 128; x pre-transposed + bf16 on
the host; output written as outT [col, batch] bf16 and transposed /
upcast on the host):
  1. phase1: layers 0..8 fused into dense 128x128 bf16 block matrices,
     with layer 9's self-scale A9 and the composed L10*L11 self-scale
     P[t] = A11[t]*A10[t] folded in on the host.  4 matmuls per tile
     into quad PSUM tiles (4 banks, 4 class-major tiles each).
  2. One wide ACT evac per quad -> E, then layer 9's partner term as a
     5th accumulating matmul per tile: psum[t] += diag(r9[t]) @ E[t^4]
     (t^4 lives in the same quad).  Second wide evac -> C1.
  3. L10/L11 partner terms as DVE scalar_tensor_tensor on the
     P-rescaled ratio chain:
       C2[u]  = C1[u] + r10[u] * C1[u^8]      (u^8 in the same quad)
       out[t] = C2[t] + r11[t] * C2[t^16]     (t^16 in the other quad)
  4. Tiles are processed class-major (t mod 4) so each class's chain
     and 1MB output DMA complete progressively.
"""
import os
import numpy as np

N = 4096
BATCH = 4096
NLAYERS = 12
NCORES = 8
BSH = BATCH // NCORES      # 512 batch rows per core
T = N // 128               # 32 column tiles

# class-major tile order: all t=c (mod 4) for c in 0..3
TILE_ORDER = [c + 4 * g for c in range(4) for g in range(8)]

_PROGRAM_CACHE = {}
LAST_EXEC_NS = None


def _bf16(a):
    import ml_dtypes
    return np.asarray(a, dtype=ml_dtypes.bfloat16)


def _extract_ab(values, idx_in, idx_out):
    """Per-layer butterfly coefficients A[k], B[k] (float64 [L, N])."""
    v = np.asarray(values, np.float64)
    ii = np.asarray(idx_in, np.int64)
    io = np.asarray(idx_out, np.int64)
    L, nnz = v.shape
    n = nnz // 2
    A = np.zeros((L, n))
    B = np.zeros((L, n))
    for k in range(L):
        s = 1 << k
        self_m = ii[k] == io[k]
        part_m = ii[k] == (io[k] ^ s)
        if not np.all(self_m | part_m):
            raise ValueError(f"layer {k}: unexpected sparse index structure")
        np.add.at(A[k], io[k][self_m], v[k][self_m])
        np.add.at(B[k], io[k][part_m], v[k][part_m])
    return A, B


def _host_precompute(values, idx_in, idx_out):
    A, B = _extract_ab(values, idx_in, idx_out)
    Ab = A.reshape(NLAYERS, T, 128)
    Bb = B.reshape(NLAYERS, T, 128)
    j = np.arange(128)

    # Block-level composition of layers 0..8: S[t] = {src_tile: 128x128}.
    S = [{t: np.eye(128)} for t in range(T)]
    for k in range(7):  # within-block layers
        s = 1 << k
        for t in range(T):
            W = np.zeros((128, 128))
            W[j, j] = Ab[k, t]
            W[j, j ^ s] = Bb[k, t]
            S[t] = {src: W @ M for src, M in S[t].items()}
    for k in (7, 8):   # cross-block layers, tile distance d
        d = 1 << (k - 7)
        newS = []
        for t in range(T):
            out = {}
            for src, M in S[t].items():
                out[src] = Ab[k, t][:, None] * M
            for src, M in S[t ^ d].items():
                out[src] = out.get(src, 0) + Bb[k, t][:, None] * M
            newS.append(out)
        S = newS

    def guard(v):
        return np.where(np.abs(v) < 1e-12, 1e-12, v)

    A9 = guard(Ab[9])
    A10 = guard(Ab[10])
    A11 = guard(Ab[11])
    # P[t] = A11[t]*A10[t]: composed L10/L11 self-scale, folded into mst
    P = A11 * A10                                   # [T, 128]

    # mst packed in class-major tile order, bf16:
    # slice k (tile t=TILE_ORDER[k]) holds 4 transposed stationaries,
    # output columns pre-scaled by A9[t]*P[t].
    mst = np.zeros((128, T * 512), np.float64)
    for k, t in enumerate(TILE_ORDER):
        assert set(S[t].keys()) == {t, t ^ 1, t ^ 2, t ^ 3}
        for ji in range(4):
            M = (P[t] * A9[t])[:, None] * S[t][t ^ ji]
            mst[:, k * 512 + ji * 128: k * 512 + (ji + 1) * 128] = M.T

    # Ratio scalars for the P-rescaled chain:
    xt = lambda v, d: v[np.arange(T) ^ d]           # tile-XOR view
    r9 = P * (Bb[9] / xt(A9, 4)) / xt(P, 4)
    r10 = P * (Bb[10] / A10) / xt(P, 8)
    r11 = P * (Bb[11] * xt(A10, 16) / (A11 * A10)) / xt(P, 16)

    # diag(r9[t]) stationaries, class-major packing [128, 32*128] bf16
    rdiag = np.zeros((128, T * 128), np.float64)
    for k, t in enumerate(TILE_ORDER):
        rdiag[j, k * 128 + j] = r9[t]

    # consts [128, 64] fp32: cols 0-31 r10, 32-63 r11
    consts = np.zeros((128, 64), np.float32)
    consts[:, 0:32] = r10.T
    consts[:, 32:64] = r11.T
    return _bf16(mst), _bf16(rdiag), consts


def _build_program():
    import concourse.bass as bass
    import concourse.mybir as mybir
    import concourse.tile as tile
    from concourse import bacc

    f32 = mybir.dt.float32
    bf16 = mybir.dt.bfloat16
    mult = mybir.AluOpType.mult
    add = mybir.AluOpType.add

    nc = bacc.Bacc("TRN2", target_bir_lowering=False, debug=False)
    # x pre-transposed + bf16 on host: [N, BSH] (column-major over batch)
    xT_ap = nc.dram_tensor("xT", [N, BSH], bf16, kind="ExternalInput").ap()
    mst_ap = nc.dram_tensor("mst", [128, T * 512], bf16, kind="ExternalInput").ap()
    rd_ap = nc.dram_tensor("rdiag", [128, T * 128], bf16, kind="ExternalInput").ap()
    cst_ap = nc.dram_tensor("consts", [128, 64], f32, kind="ExternalInput").ap()
    out_ap = nc.dram_tensor("outT", [N, BSH], bf16, kind="ExternalOutput").ap()

    with tile.TileContext(nc) as tc:
        with (
            tc.tile_pool(name="const", bufs=1) as constp,
            tc.tile_pool(name="h0", bufs=8) as h0p,
            tc.tile_pool(name="mstp", bufs=4) as mstp,
            tc.tile_pool(name="rdp", bufs=2) as rdp,
            tc.tile_pool(name="E", bufs=3) as Ep,
            tc.tile_pool(name="C1", bufs=3) as C1p,
            tc.tile_pool(name="C2", bufs=3) as C2p,
            tc.tile_pool(name="ost", bufs=3) as ostp,
            tc.tile_pool(name="ps", bufs=2, space="PSUM") as psp,
        ):
            cst = constp.tile([128, 64], f32)
            nc.sync.dma_start(cst[:], cst_ap[:])
            r10_sc = cst[:, 0:32]
            r11_sc = cst[:, 32:64]

            # mst alone on the scalar HWDGE ring (Q10); H0 + rdiag on the
            # sync ring (Q1); outputs go out via SWDGE (gpsimd) so the
            # input queues are never starved by output traffic.
            msts = {}
            for mb in range(8):
                mt = mstp.tile([128, 2048], bf16, tag="mst", name=f"mst_{mb}")
                nc.scalar.dma_start(mt[:], mst_ap[:, mb * 2048:(mb + 1) * 2048])
                msts[mb] = mt

            # H0 tiles via 512KB 3D-strided DMAs, resident for all passes:
            # H0cat[kb][p, lt*512+b] = xT[(4*kb+lt)*128 + p, b]
            H0cat = {}
            for kb in range(8):
                h0c = h0p.tile([128, 2048], bf16, tag="h0", name=f"h0c_{kb}")
                src = xT_ap[kb * 512:(kb + 1) * 512, :].rearrange(
                    "(lt p) b -> p lt b", lt=4, p=128
                )
                nc.sync.dma_start(h0c[:].rearrange("p (lt b) -> p lt b", lt=4), src)
                H0cat[kb] = h0c
            # diag(r9) stationaries in 2 chunks (512KB each)
            rds = {}
            for rb in range(2):
                rt = rdp.tile([128, 2048], bf16, tag="rd", name=f"rd_{rb}")
                nc.sync.dma_start(rt[:], rd_ap[:, rb * 2048:(rb + 1) * 2048])
                rds[rb] = rt

            E, C1, C2 = {}, {}, {}
            C2q = {}
            for k4 in range(8):          # quad index over TILE_ORDER
                quad = TILE_ORDER[k4 * 4:(k4 + 1) * 4]
                p1q = psp.tile([128, 2048], f32, name=f"p1q_{k4}", tag="ps")
                for kk, t in enumerate(quad):
                    qt, lt = t // 4, t % 4
                    mchunk = msts[(k4 * 4 + kk) // 4]
                    moff = kk * 512
                    for ji in range(4):
                        nc.tensor.matmul(
                            p1q[:, kk * 512:(kk + 1) * 512],
                            mchunk[:, moff + ji * 128: moff + (ji + 1) * 128],
                            H0cat[qt][:, (lt ^ ji) * 512:((lt ^ ji) + 1) * 512],
                            start=(ji == 0), stop=False,
                        )
                # wide ACT evac of the phase-1 results (pre-L9 state)
                eq = Ep.tile([128, 2048], bf16, tag="E", name=f"Eq_{k4}")
                nc.scalar.copy(eq[:], p1q[:])
                for kk, t in enumerate(quad):
                    E[t] = eq[:, kk * 512:(kk + 1) * 512]
                # L9 partner: psum[t] += diag(r9[t]) @ E[t^4], same quad
                # (in-quad position of t^4 is kk^1)
                for kk, t in enumerate(quad):
                    k = k4 * 4 + kk
                    nc.tensor.matmul(
                        p1q[:, kk * 512:(kk + 1) * 512],
                        rds[k // 16][:, (k % 16) * 128:(k % 16 + 1) * 128],
                        E[t ^ 4],
                        start=False, stop=True,
                    )
                # L10 with one PSUM operand per op (stt can read only one
                # non-scalar input from PSUM): evac the quad's upper half
                # (in-quad slices 2,3 = the L10 partners of slices 0,1) to
                # SBUF, then pair each psum slice with an SBUF slice.
                c1h = C1p.tile([128, 1024], bf16, tag="C1", name=f"C1h_{k4}")
                nc.vector.tensor_copy(c1h[:], p1q[:, 1024:2048])
                c2 = C2p.tile([128, 2048], bf16, tag="C2", name=f"C2q_{k4}")
                C2q[k4] = c2
                for kk, u in enumerate(quad):
                    C2[u] = c2[:, kk * 512:(kk + 1) * 512]
                    pp = kk ^ 2  # in-quad position of the L10 partner u^8
                    if kk < 2:
                        part = c1h[:, (pp - 2) * 512:(pp - 1) * 512]
                        self_ = p1q[:, kk * 512:(kk + 1) * 512]
                    else:
                        part = p1q[:, pp * 512:(pp + 1) * 512]
                        self_ = c1h[:, (kk - 2) * 512:(kk - 1) * 512]
                    nc.vector.scalar_tensor_tensor(
                        C2[u], part, r10_sc[:, u:u + 1], self_,
                        op0=mult, op1=add,
                    )

                cls, g = quad[-1] % 4, quad[-1] // 4
                if g == 7:
                    # both quads of this class done: L11 + output
                    # (two half-DMAs via SWDGE so the first half ships early)
                    tls = [cls + 4 * gg for gg in range(8)]
                    oc = ostp.tile([128, 4096], bf16, tag="ost", name=f"ost_{cls}")
                    dst = out_ap.rearrange(
                        "(g c p) b -> p c g b", g=8, c=4, p=128
                    )[:, cls, :, :]
                    ocv = oc[:].rearrange("p (g b) -> p g b", g=8)
                    for gg, tt in enumerate(tls):
                        nc.vector.scalar_tensor_tensor(
                            oc[:, gg * 512:(gg + 1) * 512],
                            C2[tt ^ 16], r11_sc[:, tt:tt + 1], C2[tt],
                            op0=mult, op1=add,
                        )
                        if gg == 3:
                            nc.gpsimd.dma_start(dst[:, 0:4, :], ocv[:, 0:4, :])
                    nc.gpsimd.dma_start(dst[:, 4:8, :], ocv[:, 4:8, :])

    nc.compile()
    return nc


def kernel(x, values, idx_in, idx_out):
    global LAST_EXEC_NS
    from concourse.bass_utils import run_bass_kernel_spmd

    x = np.asarray(x, np.float32)
    assert x.shape == (BATCH, N), x.shape
    mst, rdiag, consts = _host_precompute(values, idx_in, idx_out)
    xT = _bf16(np.ascontiguousarray(x.T))

    if "prog" not in _PROGRAM_CACHE:
        _PROGRAM_CACHE["prog"] = _build_program()
    nc = _PROGRAM_CACHE["prog"]

    in_maps = [
        {
            "xT": np.ascontiguousarray(xT[:, i * BSH:(i + 1) * BSH]),
            "mst": mst,
            "rdiag": rdiag,
            "consts": consts,
        }
        for i in range(NCORES)
    ]
    res = run_bass_kernel_spmd(nc, in_maps, core_ids=list(range(NCORES)))
    if os.environ.get("BENES_TRACE"):
        tres = run_bass_kernel_spmd(
            nc, in_maps, core_ids=list(range(NCORES)), trace=True
        )
        LAST_EXEC_NS = tres.exec_time_ns
        _PROGRAM_CACHE["profile_json"] = tres.profile_json
    out = np.empty((BATCH, N), np.float32)
    for i in range(NCORES):
        out[i * BSH:(i + 1) * BSH] = np.asarray(
            res.results[i]["outT"], np.float32
        ).T
    return out
